# revision 37
# baseline (speedup 1.0000x reference)
"""Multi-head causal attention (B=4,S=2048,D=1024,H=16,d=64) on 8 trn2 cores.

Sharding: 8 cores = 4 batches x 2 head-halves (tensor parallel over heads).
Each core handles one batch, 8 heads (4 head-pairs), and ALL 2048 queries.
K/V/Q projections are computed only for the core's own heads, so nothing is
duplicated across cores (the seq-split alternative projects full K/V twice
per batch).  The output projection is row-sharded: each core emits a partial
y^T = Wo^T[own-head rows].T-style contribution and the HOST adds the two
halves per batch (cheap numpy add; no device collectives).  The bias bo is
fed as real data to half-0 cores and zeros to half-1 cores so the host-side
add applies it exactly once.  The device program is identical on all cores;
only input data differs.

On-device structure per core:
  - phase 1: project K,Q for pair-0 chunk 0 up front; everything else is
    emitted just-in-time inside the attention stream (the PE runs its queue
    in emission order, so emission placement is the schedule).
  - phase 2: per head-pair, per 512-wide query chunk, per 128-row k-tile:
    scores^T -> exp (one fused ScalarE activation for both heads of the
    pair) -> causal mask mul on the diagonal block -> AV matmul with a
    ones-column appended to V so softmax denominators fall out of the same
    accumulator.  Remaining V/K/Q projections are interleaved into this
    stream as PE fill work so the PE never idles while ScalarE runs exp.
  - phase 3: output projection y^T = sum_p WoT_rows[p].T @ O^T[p] + bias,
    emitted per 512-query-column group as soon as pair 3's chunk for those
    columns is normalized, so it overlaps the attention tail; bf16 output
    (upcast + summed on host).

Layout tricks (kept from the seq-split version):
  - scores computed transposed S^T[sk, sq]; denominators via ones-column.
  - exp on ScalarE with 1/sqrt(64) folded into the activation scale.
  - all matmul operands bf16 (full PE rate), fp32 PSUM accumulation.
  - PSUM budget: sc [128,1024] x2 (4 banks, double-buffered so exp never
    blocks the next tile's scores) + av [128,512] x2 (2 banks,
    double-buffered across chunks) + proj acc [128,512] x2 = 8 banks.
"""

import numpy as np
import ml_dtypes

B, S, D = 4, 2048, 1024
H, DH = 16, 64
HC = H // 2            # heads per core
NP = HC // 2           # head-pairs per core = 4
NKT = S // 128         # 16 k-tiles
NDC = D // 128         # 8 contraction chunks
NCH = S // 512         # 4 query chunks of 512
VST = 66               # V column stride per head (64 V cols + 1 ones + 1 pad)

BF16 = ml_dtypes.bfloat16

_cache = {}


def _build():
    import concourse.bass as bass
    import concourse.tile as tile
    import concourse.mybir as mybir
    from concourse import bacc
    from contextlib import ExitStack

    dt = mybir.dt
    AF = mybir.ActivationFunctionType

    nc = bacc.Bacc(
        "TRN2",
        target_bir_lowering=False,
        debug=False,
        enable_asserts=False,
        num_devices=8,
    )

    qt_d = nc.dram_tensor("qT", [D, S], dt.bfloat16, kind="ExternalInput").ap()
    kt_d = nc.dram_tensor("kT", [D, S], dt.bfloat16, kind="ExternalInput").ap()
    vt_d = nc.dram_tensor("vT", [D, S], dt.bfloat16, kind="ExternalInput").ap()
    wq_d = nc.dram_tensor("Wq", [NP, 128, NDC, 2, DH], dt.bfloat16, kind="ExternalInput").ap()
    wk_d = nc.dram_tensor("Wk", [NP, 128, NDC, 2, DH], dt.bfloat16, kind="ExternalInput").ap()
    wv_d = nc.dram_tensor("Wv", [NDC, 128, HC, DH], dt.bfloat16, kind="ExternalInput").ap()
    wot_d = nc.dram_tensor("WoT", [NP, 128, D], dt.bfloat16, kind="ExternalInput").ap()
    bo_d = nc.dram_tensor("bo", [128, NDC], dt.float32, kind="ExternalInput").ap()
    mk_d = nc.dram_tensor("mask", [128, 128], dt.bfloat16, kind="ExternalInput").ap()
    y_d = nc.dram_tensor("yT", [D, S], dt.bfloat16, kind="ExternalOutput").ap()

    with tile.TileContext(nc) as tc, ExitStack() as ctx:
        const = ctx.enter_context(tc.tile_pool(name="const", bufs=1))
        work = ctx.enter_context(tc.tile_pool(name="work", bufs=2))
        pp = ctx.enter_context(tc.tile_pool(name="pp", bufs=1, space="PSUM"))

        # ---- persistent SBUF tensors ----------------------------------
        mask = const.tile([128, 128], dt.bfloat16, tag="mask")
        bo_sb = const.tile([128, NDC], dt.float32, tag="bo")
        wv_sb = const.tile([128, NDC, HC, DH], dt.bfloat16, tag="wv")
        wot_sb = [
            const.tile([128, D], dt.bfloat16, tag=f"wot{p}", name=f"wot{p}")
            for p in range(NP)
        ]
        wq_sb = [
            const.tile([128, NDC, 2, DH], dt.bfloat16, tag=f"wq{p}", name=f"wq{p}")
            for p in range(NP)
        ]
        wk_sb = [
            const.tile([128, NDC, 2, DH], dt.bfloat16, tag=f"wk{p}", name=f"wk{p}")
            for p in range(NP)
        ]
        qt_sb = [
            const.tile([128, S], dt.bfloat16, tag=f"qt{p}", name=f"qt{p}")
            for p in range(NP)
        ]
        kt_sb = [
            const.tile([128, S], dt.bfloat16, tag=f"kt{p}", name=f"kt{p}")
            for p in range(NP)
        ]
        v_sb = [
            const.tile([128, HC, VST], dt.bfloat16, tag=f"v{t}", name=f"v{t}")
            for t in range(NKT)
        ]
        ot_sb = [
            const.tile([128, S], dt.bfloat16, tag=f"ot{p}", name=f"ot{p}")
            for p in range(NP)
        ]
        warm = const.tile([1, 8], dt.float32, tag="warm")

        # persistent k-slabs: K projection fills decouple from DMA order
        ktt = [
            const.tile([128, NDC, 512], dt.bfloat16, tag=f"ktt{c}", name=f"ktt{c}")
            for c in range(NCH)
        ]

        # exp table load (~2.7us) kicked off immediately
        nc.vector.memset(warm, 0.0)
        nc.scalar.activation(out=warm, in_=warm, func=AF.Exp, scale=1.0)
        for t in range(NKT):
            nc.vector.memset(v_sb[t][:, :, 64:65], 1.0)
        # tiny dummy matmuls during the initial DMA wait finish the PE
        # p-state ramp before the first real projection arrives
        warm_ps = pp.tile([128, 512], dt.float32, tag="acc", bufs=2, name="warm_ps")
        for _ in range(40):
            nc.tensor.matmul(
                warm_ps[0:1, 0:8], lhsT=warm[:, 0:1], rhs=warm,
                start=True, stop=True,
            )

        def load_slab(src_d, bi):
            """[D, 512] D-major slab -> tt[128, dc, 512] bf16."""
            tt = work.tile([128, NDC, 512], dt.bfloat16, tag="tt", bufs=3)
            nc.sync.dma_start(
                out=tt,
                in_=src_d[:, 512 * bi : 512 * (bi + 1)].rearrange(
                    "(dc p) c -> p dc c", p=128
                ),
            )
            return tt

        def proj_unit(tt, w_sb_p, out_sb_p, bi):
            """One [128, 512] projection: out_sb_p[:, 512bi:...] = W^T @ x."""
            ps = pp.tile([128, 512], dt.float32, tag="acc", bufs=2)
            for dc in range(NDC):
                nc.tensor.matmul(
                    ps,
                    lhsT=w_sb_p[:, dc],
                    rhs=tt[:, dc, :],
                    start=(dc == 0),
                    stop=(dc == NDC - 1),
                )
            nc.vector.tensor_copy(out=out_sb_p[:, 512 * bi : 512 * (bi + 1)], in_=ps)

        def v_unit(tt, bi, tsub):
            """Project V (+ones) for k-tile 4*bi+tsub."""
            kt = 4 * bi + tsub
            ps = pp.tile([128, 512], dt.float32, tag="acc", bufs=2)
            for dc in range(NDC):
                nc.tensor.matmul(
                    ps,
                    lhsT=tt[:, dc, 128 * tsub : 128 * (tsub + 1)],
                    rhs=wv_sb[:, dc],
                    start=(dc == 0),
                    stop=(dc == NDC - 1),
                )
            nc.vector.tensor_copy(
                out=v_sb[kt][:, :, 0:DH],
                in_=ps.rearrange("p (h v) -> p h v", v=DH),
            )

        # ---- phase 0/1 + fill schedule --------------------------------
        # The PE executes its queue in emission order, so emission IS the
        # schedule.  DMA order puts pair0-chunk0's needs first; every other
        # projection unit is emitted just-in-time inside the attention
        # stream, where it doubles as PE fill work during ScalarE exp.
        def dma_wk(p):
            nc.sync.dma_start(out=wk_sb[p], in_=wk_d[p])

        def dma_ktt(c):
            nc.sync.dma_start(
                out=ktt[c],
                in_=kt_d[:, 512 * c : 512 * (c + 1)].rearrange("(dc p) c -> p dc c", p=128),
            )

        dma_ktt(0)
        dma_wk(0)
        dma_wk(1)
        q_first = load_slab(qt_d, 0)
        nc.sync.dma_start(out=wq_sb[0], in_=wq_d[0])
        v_first = load_slab(vt_d, 0)
        nc.sync.dma_start(out=wv_sb, in_=wv_d.rearrange("dc p h v -> p dc h v"))
        nc.sync.dma_start(out=mask, in_=mk_d)
        dma_ktt(1)
        # pair0-chunk0 critical path (K(0,1) fills the q-slab DMA wait)
        proj_unit(ktt[0], wk_sb[0], kt_sb[0], 0)
        proj_unit(ktt[0], wk_sb[1], kt_sb[1], 0)
        proj_unit(q_first, wq_sb[0], qt_sb[0], 0)
        # prefetched q-slabs: slabs[key] holds a loaded tile for (src, c)
        slabs = {("q", 0): q_first, ("v", 0): v_first}
        slabs[("q", 1)] = load_slab(qt_d, 1)
        for c in range(2, NCH):
            dma_ktt(c)
            dma_wk(c)
        slabs[("q", 2)] = load_slab(qt_d, 2)

        def take_slab(src, c):
            tt = slabs.pop((src, c), None)
            if tt is None:
                tt = load_slab(qt_d if src == "q" else vt_d, c)
            return tt

        def mk_prefetch(src, c):
            def go():
                if (src, c) not in slabs:
                    slabs[(src, c)] = load_slab(qt_d if src == "q" else vt_d, c)
            return go

        def mk_v_fill(c):
            def go():
                tt = take_slab("v", c)
                for tsub in range(4):
                    v_unit(tt, c, tsub)
            return go

        def mk_q_fill(c, ps):
            def go():
                tt = take_slab("q", c)
                for p in ps:
                    proj_unit(tt, wq_sb[p], qt_sb[p], c)
            return go

        def mk_k_fill(c, p):
            def go():
                proj_unit(ktt[c], wk_sb[p], kt_sb[p], c)
            return go

        def mk_wq_dma(p):
            def go():
                nc.sync.dma_start(out=wq_sb[p], in_=wq_d[p])
            return go

        fill = {(p, c): [] for p in range(NP) for c in range(NCH)}
        # pair 0: its own K/Q chunks just-in-time + all V projections
        fill[(0, 0)] += [mk_v_fill(0), mk_prefetch("v", 1), mk_wq_dma(1)]
        for c in range(1, NCH):
            fill[(0, c)] += [mk_k_fill(c, 0), mk_q_fill(c, [0]), mk_v_fill(c)]
            if c + 1 < NCH:
                fill[(0, c)].append(mk_prefetch("v", c + 1))
        fill[(0, 2)].append(mk_wq_dma(2))
        fill[(0, 3)].append(mk_prefetch("q", 0))   # reload for pair1/2
        # pair 1: K just-in-time; Q for pairs 1+2 (slab prefetched 1 chunk out)
        for c in range(NCH):
            ks = [] if c == 0 else [mk_k_fill(c, 1)]
            fill[(1, c)] += ks + [mk_q_fill(c, [1, 2])]
            if c + 1 < NCH:
                fill[(1, c)].append(mk_prefetch("q", c + 1))
        fill[(1, 2)].append(mk_wq_dma(3))
        fill[(1, 3)].append(mk_prefetch("q", 0))   # reload for pair3
        # pair 2: K just-in-time; Q for pair 3
        for c in range(NCH):
            fill[(2, c)] += [mk_k_fill(c, 2), mk_q_fill(c, [3])]
            if c + 1 < NCH:
                fill[(2, c)].append(mk_prefetch("q", c + 1))
        # pair 3: its K just-in-time (+ outproj hooks elsewhere)
        for c in range(NCH):
            fill[(3, c)].append(mk_k_fill(c, 3))

        def phase3_dmas():
            nc.sync.dma_start(out=bo_sb, in_=bo_d)
            for p in range(NP):
                nc.sync.dma_start(out=wot_sb[p], in_=wot_d[p])

        fill[(3, 0)].append(phase3_dmas)

        # ---- phase 2: attention ---------------------------------------
        def emit_av(p, avs, prev, c, ntile):
            t, pt = prev
            start = max(0, 128 * t - 512 * c)
            for s in range(2):
                nc.tensor.matmul(
                    avs[s][0:65, start:512],
                    lhsT=v_sb[t][:, 2 * p + s, 0:65],
                    rhs=pt[:, s, start:512],
                    start=(t == 0),
                    stop=(t == ntile - 1),
                )

        pending_norm = [None]

        def mk_norm(p, c, avs):
            def go():
                for s in range(2):
                    po = 64 * s
                    # reciprocal of the denominator row straight out of PSUM,
                    # broadcast to 64 partitions on the (otherwise idle)
                    # GPSIMD engine; the PE stays out of the normalize.
                    rd = work.tile([1, 512], dt.float32, tag="rd", bufs=4)
                    nc.vector.reciprocal(out=rd, in_=avs[s][64:65, :])
                    rb = work.tile([64, 512], dt.float32, tag="rb", bufs=4)
                    nc.gpsimd.partition_broadcast(rb, rd)
                    nc.vector.tensor_mul(
                        ot_sb[p][po : po + 64, 512 * c : 512 * (c + 1)],
                        avs[s][0:64, :],
                        rb,
                    )
            return go

        def outproj_hf(hf):
            """Output projection for query columns [512*hf, 512*hf+512).
            Emitted as soon as every pair's chunk-hf normalize is done, so
            phase 3 overlaps the tail of the attention stream."""
            c0 = 512 * hf
            for dc in range(NDC):
                yp = pp.tile([128, 512], dt.float32, tag="acc", bufs=2)
                for p in range(NP):
                    nc.tensor.matmul(
                        yp,
                        lhsT=wot_sb[p][:, 128 * dc : 128 * (dc + 1)],
                        rhs=ot_sb[p][:, c0 : c0 + 512],
                        start=(p == 0),
                        stop=(p == NP - 1),
                    )
                ys = work.tile([128, 512], dt.bfloat16, tag="ys", bufs=3)
                nc.vector.tensor_scalar_add(ys, yp, bo_sb[:, dc : dc + 1])
                nc.sync.dma_start(
                    out=y_d[128 * dc : 128 * (dc + 1), c0 : c0 + 512], in_=ys
                )

        for p in range(NP):
            for c in range(NCH):
                ntile = 4 * c + 4          # k-tiles 0..4c+3
                for th in fill[(p, c)]:
                    th()
                # previous chunk's bcast+recip+mul, behind the fills so the
                # broadcast matmul runs on a warm PE
                if pending_norm[0] is not None:
                    pending_norm[0]()
                if p == NP - 1 and c >= 1:
                    outproj_hf(c - 1)
                avs = [
                    pp.tile([128, 512], dt.float32, tag="av", bufs=2, name=f"av{s}")
                    for s in range(2)
                ]

                prev = None
                for t in range(ntile):
                    start = max(0, 128 * t - 512 * c)
                    # scores^T for both heads into one PSUM tile
                    sc = pp.tile([128, 1024], dt.float32, tag="sc", bufs=2)
                    for s in range(2):
                        po = 64 * s
                        nc.tensor.matmul(
                            sc[:, 512 * s + start : 512 * (s + 1)],
                            lhsT=kt_sb[p][po : po + 64, 128 * t : 128 * (t + 1)],
                            rhs=qt_sb[p][po : po + 64, 512 * c + start : 512 * (c + 1)],
                            start=True,
                            stop=True,
                        )
                    # fused exp for both heads
                    pt = work.tile([128, 2, 512], dt.bfloat16, tag="pt", bufs=4)
                    nc.scalar.activation(
                        out=pt[:, :, start:512],
                        in_=sc.rearrange("p (s l) -> p s l", s=2)[:, :, start:512],
                        func=AF.Exp,
                        scale=0.125,
                    )
                    if start > 0 or t == 4 * c:
                        # diagonal tile: mask the leading 128 columns
                        for s in range(2):
                            nc.vector.tensor_mul(
                                pt[:, s, start : start + 128],
                                pt[:, s, start : start + 128],
                                mask,
                            )
                    if prev is not None:
                        emit_av(p, avs, prev, c, ntile)
                    prev = (t, pt)
                emit_av(p, avs, prev, c, ntile)

                pending_norm[0] = mk_norm(p, c, avs)

        # final column group: pairs 0-2 pre-accumulated during pair3-c3
        # attention (started above); only pair 3's matmul + bias + dma wait
        # for the last normalize.
        pending_norm[0]()
        outproj_hf(NCH - 1)

    nc.compile()
    return nc


def _get_program():
    if "nc" not in _cache:
        _cache["nc"] = _build()
    return _cache["nc"]


def kernel(q, k, v, Wq, Wk, Wv, Wo, bo, trace=False):
    from concourse.bass_utils import run_bass_kernel_spmd

    nc = _get_program()
    in_maps = _make_in_maps(q, k, v, Wq, Wk, Wv, Wo, bo)
    res = run_bass_kernel_spmd(nc, in_maps, core_ids=list(range(8)), trace=trace)
    _cache["last_results"] = res

    out = np.empty((B, S, D), np.float32)
    for b in range(B):
        out[b] = (
            res.results[2 * b]["yT"].astype(np.float32)
            + res.results[2 * b + 1]["yT"].astype(np.float32)
        ).T
    return out


def last_exec_time_ns():
    res = _cache.get("last_results")
    return getattr(res, "exec_time_ns", None) if res is not None else None


def _make_in_maps(q, k, v, Wq, Wk, Wv, Wo, bo):
    q = np.asarray(q, np.float32)
    k = np.asarray(k, np.float32)
    v = np.asarray(v, np.float32)

    def _pack_qk(W, g):
        # [H, D, DH] half-slice -> [NP, 128, NDC, 2, DH] (2KB DMA runs)
        Wg = np.asarray(W, np.float32)[8 * g : 8 * (g + 1)].astype(BF16)
        return np.ascontiguousarray(
            Wg.reshape(NP, 2, NDC, 128, DH).transpose(0, 3, 2, 1, 4)
        )

    WoT = np.ascontiguousarray(np.asarray(Wo, np.float32).T).astype(BF16)
    bo_fp = np.ascontiguousarray(np.asarray(bo, np.float32).reshape(NDC, 128).T)
    mask = np.triu(np.ones((128, 128), np.float32)).astype(BF16)

    halves = []
    for g in range(2):
        halves.append(
            {
                "Wq": _pack_qk(Wq, g),
                "Wk": _pack_qk(Wk, g),
                "Wv": np.ascontiguousarray(
                    np.asarray(Wv, np.float32)[8 * g : 8 * (g + 1)]
                    .astype(BF16)
                    .transpose(1, 0, 2)
                    .reshape(NDC, 128, HC, DH)
                ),
                "WoT": np.ascontiguousarray(
                    WoT[512 * g : 512 * (g + 1)].reshape(NP, 128, D)
                ),
                "bo": bo_fp if g == 0 else np.zeros_like(bo_fp),
                "mask": mask,
            }
        )

    in_maps = []
    for b in range(B):
        qT = np.ascontiguousarray(q[b].T).astype(BF16)
        kT = np.ascontiguousarray(k[b].T).astype(BF16)
        vT = np.ascontiguousarray(v[b].T).astype(BF16)
        for g in range(2):
            in_maps.append({"qT": qT, "kT": kT, "vT": vT, **halves[g]})
    return in_maps


def benchmark(q, k, v, Wq, Wk, Wv, Wo, bo, iters=20):
    """Steady-state device timing: jit once, keep inputs device-resident,
    time repeated executions.  Returns (per_iter_seconds_list, output)."""
    import time
    import jax
    import jax.numpy as jnp
    import concourse.mybir as mybir
    from jax.experimental.shard_map import shard_map
    from jax.sharding import Mesh, NamedSharding, PartitionSpec
    from concourse import bass2jax

    nc = _get_program()
    bass2jax.install_neuronx_cc_hook()

    in_maps = _make_in_maps(q, k, v, Wq, Wk, Wv, Wo, bo)

    partition_name = nc.partition_id_tensor.name if nc.partition_id_tensor else None
    in_names, out_names, out_avals, zero_shapes = [], [], [], []
    for alloc in nc.m.functions[0].allocations:
        if not isinstance(alloc, mybir.MemoryLocationSet):
            continue
        name = alloc.memorylocations[0].name
        if alloc.kind == "ExternalInput":
            if name != partition_name:
                in_names.append(name)
        elif alloc.kind == "ExternalOutput":
            out_names.append(name)
            shape = tuple(alloc.tensor_shape)
            dtype = mybir.dt.np(alloc.dtype)
            out_avals.append(jax.core.ShapedArray(shape, dtype))
            zero_shapes.append((shape, dtype))
    n_params = len(in_names)
    all_names = in_names + out_names
    if partition_name is not None:
        all_names.append(partition_name)
    donate = tuple(range(n_params, n_params + len(out_names)))

    n_outs = len(out_names)

    def _one(args):
        operands = list(args)
        if partition_name is not None:
            operands.append(bass2jax.partition_id_tensor())
        outs = bass2jax._bass_exec_p.bind(
            *operands,
            out_avals=tuple(out_avals),
            in_names=tuple(all_names),
            out_names=tuple(out_names),
            lowering_input_output_aliases=(),
            sim_require_finite=True,
            sim_require_nnan=True,
            nc=nc,
        )
        return tuple(outs)

    def _body(*args):
        return _one(args)

    devices = jax.devices()[:8]
    mesh = Mesh(np.asarray(devices), ("core",))
    spec = PartitionSpec("core")
    sh = NamedSharding(mesh, spec)
    f1 = jax.jit(
        shard_map(
            _body, mesh=mesh,
            in_specs=(spec,) * (n_params + n_outs),
            out_specs=(spec,) * n_outs,
            check_rep=False,
        ),
        donate_argnums=donate,
        keep_unused=True,
    )
    concat_in = [
        jax.device_put(
            np.concatenate([np.asarray(in_maps[c][nm]) for c in range(8)], axis=0), sh
        )
        for nm in in_names
    ]

    zfns = [
        jax.jit(
            (lambda s, d: (lambda: jnp.zeros((8 * s[0], *s[1:]), d)))(s, d),
            out_shardings=sh,
        )
        for s, d in zero_shapes
    ]

    def make_zeros(n):
        return [[zf() for zf in zfns] for _ in range(n)]

    # warmup (compile)
    out_arrs = f1(*concat_in, *make_zeros(1)[0])
    jax.block_until_ready(out_arrs)

    # slope fit across chain depths, robust to bimodal dispatch latency
    depths = [4, 16, 40]
    samples = {d: [] for d in depths}
    for _ in range(max(iters, 14)):
        for d in depths:
            zsl = make_zeros(d)
            jax.block_until_ready(zsl)
            t0 = time.perf_counter()
            outs = [f1(*concat_in, *zsl[i]) for i in range(d)]
            jax.block_until_ready(outs)
            samples[d].append(time.perf_counter() - t0)
            out_arrs = outs[-1]
    mins = {d: min(v) for d, v in samples.items()}
    slopes = [
        (mins[d2] - mins[d1]) / (d2 - d1)
        for i, d1 in enumerate(depths)
        for d2 in depths[i + 1 :]
        if mins[d2] > mins[d1]
    ]
    per_exec = float(min(slopes)) if slopes else float("nan")
    t1s = samples[depths[0]]
    _cache["bench"] = {
        "t1": float(mins[depths[0]]),
        "tN": float(mins[depths[-1]]),
        "chain": depths[-1],
        "per_exec": per_exec,
        "mins": mins,
    }

    out = np.empty((B, S, D), np.float32)
    yT_all = np.asarray(out_arrs[out_names.index("yT")]).reshape(8, D, S)
    for b in range(B):
        out[b] = (yT_all[2 * b].astype(np.float32) + yT_all[2 * b + 1].astype(np.float32)).T
    return t1s, out


# revision 38
# speedup vs baseline: 1.0159x; 1.0159x over previous
"""Multi-head causal attention (B=4,S=2048,D=1024,H=16,d=64) on 8 trn2 cores.

Sharding: 8 cores = 4 batches x 2 head-halves (tensor parallel over heads).
Each core handles one batch, 8 heads (4 head-pairs), and ALL 2048 queries.
K/V/Q projections are computed only for the core's own heads, so nothing is
duplicated across cores (the seq-split alternative projects full K/V twice
per batch).  The output projection is row-sharded: each core emits a partial
y^T = Wo^T[own-head rows].T-style contribution and the HOST adds the two
halves per batch (cheap numpy add; no device collectives).  The bias bo is
fed as real data to half-0 cores and zeros to half-1 cores so the host-side
add applies it exactly once.  The device program is identical on all cores;
only input data differs.

On-device structure per core:
  - phase 1: project K,Q for pair-0 chunk 0 up front; everything else is
    emitted just-in-time inside the attention stream (the PE runs its queue
    in emission order, so emission placement is the schedule).
  - phase 2: per head-pair, per 512-wide query chunk, per 128-row k-tile:
    scores^T -> exp (one fused ScalarE activation for both heads of the
    pair) -> causal mask mul on the diagonal block -> AV matmul with a
    ones-column appended to V so softmax denominators fall out of the same
    accumulator.  Remaining V/K/Q projections are interleaved into this
    stream as PE fill work so the PE never idles while ScalarE runs exp.
  - phase 3: output projection y^T = sum_p WoT_rows[p].T @ O^T[p] + bias,
    emitted per 512-query-column group as soon as pair 3's chunk for those
    columns is normalized, so it overlaps the attention tail; bf16 output
    (upcast + summed on host).

Layout tricks (kept from the seq-split version):
  - scores computed transposed S^T[sk, sq]; denominators via ones-column.
  - exp on ScalarE with 1/sqrt(64) folded into the activation scale.
  - all matmul operands bf16 (full PE rate), fp32 PSUM accumulation.
  - PSUM budget: sc [128,1024] x2 (4 banks, double-buffered so exp never
    blocks the next tile's scores) + av [128,512] x2 (2 banks,
    double-buffered across chunks) + proj acc [128,512] x2 = 8 banks.
"""

import numpy as np
import ml_dtypes

B, S, D = 4, 2048, 1024
H, DH = 16, 64
HC = H // 2            # heads per core
NP = HC // 2           # head-pairs per core = 4
NKT = S // 128         # 16 k-tiles
NDC = D // 128         # 8 contraction chunks
NCH = S // 512         # 4 query chunks of 512
VST = 66               # V column stride per head (64 V cols + 1 ones + 1 pad)

BF16 = ml_dtypes.bfloat16

_cache = {}


def _build():
    import concourse.bass as bass
    import concourse.tile as tile
    import concourse.mybir as mybir
    from concourse import bacc
    from contextlib import ExitStack

    dt = mybir.dt
    AF = mybir.ActivationFunctionType

    nc = bacc.Bacc(
        "TRN2",
        target_bir_lowering=False,
        debug=False,
        enable_asserts=False,
        num_devices=8,
    )

    qt_d = nc.dram_tensor("qT", [D, S], dt.bfloat16, kind="ExternalInput").ap()
    kt_d = nc.dram_tensor("kT", [D, S], dt.bfloat16, kind="ExternalInput").ap()
    vt_d = nc.dram_tensor("vT", [D, S], dt.bfloat16, kind="ExternalInput").ap()
    wq_d = nc.dram_tensor("Wq", [NP, 128, NDC, 2, DH], dt.bfloat16, kind="ExternalInput").ap()
    wk_d = nc.dram_tensor("Wk", [NP, 128, NDC, 2, DH], dt.bfloat16, kind="ExternalInput").ap()
    wv_d = nc.dram_tensor("Wv", [NDC, 128, HC, DH], dt.bfloat16, kind="ExternalInput").ap()
    wot_d = nc.dram_tensor("WoT", [NP, 128, D], dt.bfloat16, kind="ExternalInput").ap()
    bo_d = nc.dram_tensor("bo", [128, NDC], dt.float32, kind="ExternalInput").ap()
    mk_d = nc.dram_tensor("mask", [128, 128], dt.bfloat16, kind="ExternalInput").ap()
    y_d = nc.dram_tensor("yT", [D, S], dt.bfloat16, kind="ExternalOutput").ap()

    with tile.TileContext(nc) as tc, ExitStack() as ctx:
        const = ctx.enter_context(tc.tile_pool(name="const", bufs=1))
        work = ctx.enter_context(tc.tile_pool(name="work", bufs=2))
        pp = ctx.enter_context(tc.tile_pool(name="pp", bufs=1, space="PSUM"))

        # ---- persistent SBUF tensors ----------------------------------
        mask = const.tile([128, 128], dt.bfloat16, tag="mask")
        bo_sb = const.tile([128, NDC], dt.float32, tag="bo")
        wv_sb = const.tile([128, NDC, HC, DH], dt.bfloat16, tag="wv")
        wot_sb = [
            const.tile([128, D], dt.bfloat16, tag=f"wot{p}", name=f"wot{p}")
            for p in range(NP)
        ]
        wq_sb = [
            const.tile([128, NDC, 2, DH], dt.bfloat16, tag=f"wq{p}", name=f"wq{p}")
            for p in range(NP)
        ]
        wk_sb = [
            const.tile([128, NDC, 2, DH], dt.bfloat16, tag=f"wk{p}", name=f"wk{p}")
            for p in range(NP)
        ]
        qt_sb = [
            const.tile([128, S], dt.bfloat16, tag=f"qt{p}", name=f"qt{p}")
            for p in range(NP)
        ]
        kt_sb = [
            const.tile([128, S], dt.bfloat16, tag=f"kt{p}", name=f"kt{p}")
            for p in range(NP)
        ]
        v_sb = [
            const.tile([128, HC, VST], dt.bfloat16, tag=f"v{t}", name=f"v{t}")
            for t in range(NKT)
        ]
        ot_sb = [
            const.tile([128, S], dt.bfloat16, tag=f"ot{p}", name=f"ot{p}")
            for p in range(NP)
        ]
        warm = const.tile([1, 8], dt.float32, tag="warm")

        # persistent k-slabs: K projection fills decouple from DMA order
        ktt = [
            const.tile([128, NDC, 512], dt.bfloat16, tag=f"ktt{c}", name=f"ktt{c}")
            for c in range(NCH)
        ]

        # exp table load (~2.7us) kicked off immediately
        nc.vector.memset(warm, 0.0)
        nc.scalar.activation(out=warm, in_=warm, func=AF.Exp, scale=1.0)
        for t in range(NKT):
            nc.vector.memset(v_sb[t][:, :, 64:65], 1.0)
        # tiny dummy matmuls during the initial DMA wait finish the PE
        # p-state ramp before the first real projection arrives
        warm_ps = pp.tile([128, 512], dt.float32, tag="acc", bufs=2, name="warm_ps")
        for _ in range(40):
            nc.tensor.matmul(
                warm_ps[0:1, 0:8], lhsT=warm[:, 0:1], rhs=warm,
                start=True, stop=True,
            )

        def load_slab(src_d, bi):
            """[D, 512] D-major slab -> tt[128, dc, 512] bf16."""
            tt = work.tile([128, NDC, 512], dt.bfloat16, tag="tt", bufs=3)
            nc.sync.dma_start(
                out=tt,
                in_=src_d[:, 512 * bi : 512 * (bi + 1)].rearrange(
                    "(dc p) c -> p dc c", p=128
                ),
            )
            return tt

        def proj_unit(tt, w_sb_p, out_sb_p, bi):
            """One [128, 512] projection: out_sb_p[:, 512bi:...] = W^T @ x."""
            ps = pp.tile([128, 512], dt.float32, tag="acc", bufs=2)
            for dc in range(NDC):
                nc.tensor.matmul(
                    ps,
                    lhsT=w_sb_p[:, dc],
                    rhs=tt[:, dc, :],
                    start=(dc == 0),
                    stop=(dc == NDC - 1),
                )
            nc.vector.tensor_copy(out=out_sb_p[:, 512 * bi : 512 * (bi + 1)], in_=ps)

        def v_unit(tt, bi, tsub):
            """Project V (+ones) for k-tile 4*bi+tsub."""
            kt = 4 * bi + tsub
            ps = pp.tile([128, 512], dt.float32, tag="acc", bufs=2)
            for dc in range(NDC):
                nc.tensor.matmul(
                    ps,
                    lhsT=tt[:, dc, 128 * tsub : 128 * (tsub + 1)],
                    rhs=wv_sb[:, dc],
                    start=(dc == 0),
                    stop=(dc == NDC - 1),
                )
            nc.vector.tensor_copy(
                out=v_sb[kt][:, :, 0:DH],
                in_=ps.rearrange("p (h v) -> p h v", v=DH),
            )

        # ---- phase 0/1 + fill schedule --------------------------------
        # The PE executes its queue in emission order, so emission IS the
        # schedule.  DMA order puts pair0-chunk0's needs first; every other
        # projection unit is emitted just-in-time inside the attention
        # stream, where it doubles as PE fill work during ScalarE exp.
        def dma_wk(p):
            nc.sync.dma_start(out=wk_sb[p], in_=wk_d[p])

        def dma_ktt(c):
            nc.sync.dma_start(
                out=ktt[c],
                in_=kt_d[:, 512 * c : 512 * (c + 1)].rearrange("(dc p) c -> p dc c", p=128),
            )

        dma_ktt(0)
        dma_wk(0)
        dma_wk(1)
        q_first = load_slab(qt_d, 0)
        nc.sync.dma_start(out=wq_sb[0], in_=wq_d[0])
        v_first = load_slab(vt_d, 0)
        nc.sync.dma_start(out=wv_sb, in_=wv_d.rearrange("dc p h v -> p dc h v"))
        nc.sync.dma_start(out=mask, in_=mk_d)
        dma_ktt(1)
        # pair0-chunk0 critical path (K(0,1) fills the q-slab DMA wait)
        proj_unit(ktt[0], wk_sb[0], kt_sb[0], 0)
        proj_unit(ktt[0], wk_sb[1], kt_sb[1], 0)
        proj_unit(q_first, wq_sb[0], qt_sb[0], 0)
        # prefetched q-slabs: slabs[key] holds a loaded tile for (src, c)
        slabs = {("q", 0): q_first, ("v", 0): v_first}
        slabs[("q", 1)] = load_slab(qt_d, 1)
        for c in range(2, NCH):
            dma_ktt(c)
            dma_wk(c)
        slabs[("q", 2)] = load_slab(qt_d, 2)

        def take_slab(src, c):
            tt = slabs.pop((src, c), None)
            if tt is None:
                tt = load_slab(qt_d if src == "q" else vt_d, c)
            return tt

        def mk_prefetch(src, c):
            def go():
                if (src, c) not in slabs:
                    slabs[(src, c)] = load_slab(qt_d if src == "q" else vt_d, c)
            return go

        def mk_v_fill(c):
            def go():
                tt = take_slab("v", c)
                for tsub in range(4):
                    v_unit(tt, c, tsub)
            return go

        def mk_q_fill(c, ps):
            def go():
                tt = take_slab("q", c)
                for p in ps:
                    proj_unit(tt, wq_sb[p], qt_sb[p], c)
            return go

        def mk_k_fill(c, p):
            def go():
                proj_unit(ktt[c], wk_sb[p], kt_sb[p], c)
            return go

        def mk_wq_dma(p):
            def go():
                nc.sync.dma_start(out=wq_sb[p], in_=wq_d[p])
            return go

        fill = {(p, c): [] for p in range(NP) for c in range(NCH)}
        # pair 0: its own K/Q chunks just-in-time + all V projections
        fill[(0, 0)] += [mk_v_fill(0), mk_prefetch("v", 1), mk_wq_dma(1)]
        for c in range(1, NCH):
            fill[(0, c)] += [mk_k_fill(c, 0), mk_q_fill(c, [0]), mk_v_fill(c)]
            if c + 1 < NCH:
                fill[(0, c)].append(mk_prefetch("v", c + 1))
        fill[(0, 2)].append(mk_wq_dma(2))
        fill[(0, 3)].append(mk_prefetch("q", 0))   # reload for pair1/2
        # pair 1: K just-in-time; Q for pairs 1+2 (slab prefetched 1 chunk out)
        for c in range(NCH):
            ks = [] if c == 0 else [mk_k_fill(c, 1)]
            fill[(1, c)] += ks + [mk_q_fill(c, [1, 2])]
            if c + 1 < NCH:
                fill[(1, c)].append(mk_prefetch("q", c + 1))
        fill[(1, 2)].append(mk_wq_dma(3))
        fill[(1, 3)].append(mk_prefetch("q", 0))   # reload for pair3
        # pair 2: K just-in-time; Q for pair 3
        for c in range(NCH):
            fill[(2, c)] += [mk_k_fill(c, 2), mk_q_fill(c, [3])]
            if c + 1 < NCH:
                fill[(2, c)].append(mk_prefetch("q", c + 1))
        # pair 3: its K just-in-time (+ outproj hooks elsewhere)
        for c in range(NCH):
            fill[(3, c)].append(mk_k_fill(c, 3))

        def phase3_dmas():
            nc.sync.dma_start(out=bo_sb, in_=bo_d)
            for p in range(NP):
                nc.sync.dma_start(out=wot_sb[p], in_=wot_d[p])

        fill[(3, 0)].append(phase3_dmas)

        # ---- phase 2: attention ---------------------------------------
        def emit_av(p, avs, prev, c, ntile):
            t, pt = prev
            start = max(0, 128 * t - 512 * c)
            for s in range(2):
                nc.tensor.matmul(
                    avs[s][0:65, start:512],
                    lhsT=v_sb[t][:, 2 * p + s, 0:65],
                    rhs=pt[:, s, start:512],
                    start=(t == 0),
                    stop=(t == ntile - 1),
                )

        pending_norm = [None]

        def mk_norm(p, c, avs):
            def go():
                for s in range(2):
                    po = 64 * s
                    # reciprocal of the denominator row straight out of PSUM,
                    # broadcast to 64 partitions on the (otherwise idle)
                    # GPSIMD engine; the PE stays out of the normalize.
                    rd = work.tile([1, 512], dt.float32, tag="rd", bufs=4)
                    nc.vector.reciprocal(out=rd, in_=avs[s][64:65, :])
                    rb = work.tile([64, 512], dt.float32, tag="rb", bufs=4)
                    nc.gpsimd.partition_broadcast(rb, rd)
                    nc.vector.tensor_mul(
                        ot_sb[p][po : po + 64, 512 * c : 512 * (c + 1)],
                        avs[s][0:64, :],
                        rb,
                    )
            return go

        def outproj_hf(hf):
            """Output projection for query columns [512*hf, 512*hf+512).
            Emitted as soon as every pair's chunk-hf normalize is done, so
            phase 3 overlaps the tail of the attention stream."""
            c0 = 512 * hf
            for dc in range(NDC):
                yp = pp.tile([128, 512], dt.float32, tag="acc", bufs=2)
                for p in range(NP):
                    nc.tensor.matmul(
                        yp,
                        lhsT=wot_sb[p][:, 128 * dc : 128 * (dc + 1)],
                        rhs=ot_sb[p][:, c0 : c0 + 512],
                        start=(p == 0),
                        stop=(p == NP - 1),
                    )
                ys = work.tile([128, 512], dt.bfloat16, tag="ys", bufs=3)
                nc.vector.tensor_scalar_add(ys, yp, bo_sb[:, dc : dc + 1])
                nc.sync.dma_start(
                    out=y_d[128 * dc : 128 * (dc + 1), c0 : c0 + 512], in_=ys
                )

        for p in range(NP):
            for c in range(NCH):
                ntile = 4 * c + 4          # k-tiles 0..4c+3
                for th in fill[(p, c)]:
                    th()
                # previous chunk's bcast+recip+mul, behind the fills so the
                # broadcast matmul runs on a warm PE
                if pending_norm[0] is not None:
                    pending_norm[0]()
                if p == NP - 1 and c >= 1:
                    outproj_hf(c - 1)
                avs = [
                    pp.tile([128, 512], dt.float32, tag="av", bufs=2, name=f"av{s}")
                    for s in range(2)
                ]

                prev = None
                for t in range(ntile):
                    start = max(0, 128 * t - 512 * c)
                    # scores^T for both heads into one PSUM tile
                    sc = pp.tile([128, 1024], dt.float32, tag="sc", bufs=2)
                    for s in range(2):
                        po = 64 * s
                        nc.tensor.matmul(
                            sc[:, 512 * s + start : 512 * (s + 1)],
                            lhsT=kt_sb[p][po : po + 64, 128 * t : 128 * (t + 1)],
                            rhs=qt_sb[p][po : po + 64, 512 * c + start : 512 * (c + 1)],
                            start=True,
                            stop=True,
                        )
                    # fused exp for both heads
                    pt = work.tile([128, 2, 512], dt.bfloat16, tag="pt", bufs=4)
                    nc.scalar.activation(
                        out=pt[:, :, start:512],
                        in_=sc.rearrange("p (s l) -> p s l", s=2)[:, :, start:512],
                        func=AF.Exp,
                        scale=0.125,
                    )
                    if start > 0 or t == 4 * c:
                        # diagonal tile: mask the leading 128 columns
                        for s in range(2):
                            nc.vector.tensor_mul(
                                pt[:, s, start : start + 128],
                                pt[:, s, start : start + 128],
                                mask,
                            )
                    if prev is not None:
                        emit_av(p, avs, prev, c, ntile)
                    prev = (t, pt)
                emit_av(p, avs, prev, c, ntile)

                pending_norm[0] = mk_norm(p, c, avs)

        # final column group: pairs 0-2 pre-accumulated during pair3-c3
        # attention (started above); only pair 3's matmul + bias + dma wait
        # for the last normalize.
        pending_norm[0]()
        outproj_hf(NCH - 1)

    nc.compile()
    return nc


def _get_program():
    if "nc" not in _cache:
        _cache["nc"] = _build()
    return _cache["nc"]


def kernel(q, k, v, Wq, Wk, Wv, Wo, bo, trace=False):
    from concourse.bass_utils import run_bass_kernel_spmd

    nc = _get_program()
    in_maps = _make_in_maps(q, k, v, Wq, Wk, Wv, Wo, bo)
    res = run_bass_kernel_spmd(nc, in_maps, core_ids=list(range(8)), trace=trace)
    _cache["last_results"] = res

    out = np.empty((B, S, D), np.float32)
    for b in range(B):
        out[b] = (
            res.results[2 * b]["yT"].astype(np.float32)
            + res.results[2 * b + 1]["yT"].astype(np.float32)
        ).T
    return out


def last_exec_time_ns():
    res = _cache.get("last_results")
    return getattr(res, "exec_time_ns", None) if res is not None else None


def _make_in_maps(q, k, v, Wq, Wk, Wv, Wo, bo):
    q = np.asarray(q, np.float32)
    k = np.asarray(k, np.float32)
    v = np.asarray(v, np.float32)

    def _pack_qk(W, g):
        # [H, D, DH] half-slice -> [NP, 128, NDC, 2, DH] (2KB DMA runs)
        Wg = np.asarray(W, np.float32)[8 * g : 8 * (g + 1)].astype(BF16)
        return np.ascontiguousarray(
            Wg.reshape(NP, 2, NDC, 128, DH).transpose(0, 3, 2, 1, 4)
        )

    WoT = np.ascontiguousarray(np.asarray(Wo, np.float32).T).astype(BF16)
    bo_fp = np.ascontiguousarray(np.asarray(bo, np.float32).reshape(NDC, 128).T)
    mask = np.triu(np.ones((128, 128), np.float32)).astype(BF16)

    halves = []
    for g in range(2):
        halves.append(
            {
                "Wq": _pack_qk(Wq, g),
                "Wk": _pack_qk(Wk, g),
                "Wv": np.ascontiguousarray(
                    np.asarray(Wv, np.float32)[8 * g : 8 * (g + 1)]
                    .astype(BF16)
                    .transpose(1, 0, 2)
                    .reshape(NDC, 128, HC, DH)
                ),
                "WoT": np.ascontiguousarray(
                    WoT[512 * g : 512 * (g + 1)].reshape(NP, 128, D)
                ),
                "bo": bo_fp if g == 0 else np.zeros_like(bo_fp),
                "mask": mask,
            }
        )

    in_maps = []
    for b in range(B):
        qT = np.ascontiguousarray(q[b].T).astype(BF16)
        kT = np.ascontiguousarray(k[b].T).astype(BF16)
        vT = np.ascontiguousarray(v[b].T).astype(BF16)
        for g in range(2):
            in_maps.append({"qT": qT, "kT": kT, "vT": vT, **halves[g]})
    return in_maps


def benchmark(q, k, v, Wq, Wk, Wv, Wo, bo, iters=20):
    """Steady-state device timing: jit once, keep inputs device-resident,
    time repeated executions.  Returns (per_iter_seconds_list, output)."""
    import time
    import jax
    import jax.numpy as jnp
    import concourse.mybir as mybir
    from jax.experimental.shard_map import shard_map
    from jax.sharding import Mesh, NamedSharding, PartitionSpec
    from concourse import bass2jax

    nc = _get_program()
    bass2jax.install_neuronx_cc_hook()

    in_maps = _make_in_maps(q, k, v, Wq, Wk, Wv, Wo, bo)

    partition_name = nc.partition_id_tensor.name if nc.partition_id_tensor else None
    in_names, out_names, out_avals, zero_shapes = [], [], [], []
    for alloc in nc.m.functions[0].allocations:
        if not isinstance(alloc, mybir.MemoryLocationSet):
            continue
        name = alloc.memorylocations[0].name
        if alloc.kind == "ExternalInput":
            if name != partition_name:
                in_names.append(name)
        elif alloc.kind == "ExternalOutput":
            out_names.append(name)
            shape = tuple(alloc.tensor_shape)
            dtype = mybir.dt.np(alloc.dtype)
            out_avals.append(jax.core.ShapedArray(shape, dtype))
            zero_shapes.append((shape, dtype))
    n_params = len(in_names)
    all_names = in_names + out_names
    if partition_name is not None:
        all_names.append(partition_name)
    donate = tuple(range(n_params, n_params + len(out_names)))

    n_outs = len(out_names)

    def _one(args):
        operands = list(args)
        if partition_name is not None:
            operands.append(bass2jax.partition_id_tensor())
        outs = bass2jax._bass_exec_p.bind(
            *operands,
            out_avals=tuple(out_avals),
            in_names=tuple(all_names),
            out_names=tuple(out_names),
            lowering_input_output_aliases=(),
            sim_require_finite=True,
            sim_require_nnan=True,
            nc=nc,
        )
        return tuple(outs)

    def _body(*args):
        return _one(args)

    devices = jax.devices()[:8]
    mesh = Mesh(np.asarray(devices), ("core",))
    spec = PartitionSpec("core")
    sh = NamedSharding(mesh, spec)
    f1 = jax.jit(
        shard_map(
            _body, mesh=mesh,
            in_specs=(spec,) * (n_params + n_outs),
            out_specs=(spec,) * n_outs,
            check_rep=False,
        ),
        donate_argnums=donate,
        keep_unused=True,
    )
    concat_in = [
        jax.device_put(
            np.concatenate([np.asarray(in_maps[c][nm]) for c in range(8)], axis=0), sh
        )
        for nm in in_names
    ]

    zfns = [
        jax.jit(
            (lambda s, d: (lambda: jnp.zeros((8 * s[0], *s[1:]), d)))(s, d),
            out_shardings=sh,
        )
        for s, d in zero_shapes
    ]

    def make_zeros(n):
        return [[zf() for zf in zfns] for _ in range(n)]

    # warmup (compile)
    out_arrs = f1(*concat_in, *make_zeros(1)[0])
    jax.block_until_ready(out_arrs)

    # slope fit across chain depths, robust to bimodal dispatch latency
    depths = [4, 16, 40]
    samples = {d: [] for d in depths}
    for _ in range(max(iters, 14)):
        for d in depths:
            zsl = make_zeros(d)
            jax.block_until_ready(zsl)
            t0 = time.perf_counter()
            outs = [f1(*concat_in, *zsl[i]) for i in range(d)]
            jax.block_until_ready(outs)
            samples[d].append(time.perf_counter() - t0)
            out_arrs = outs[-1]
    mins = {d: min(v) for d, v in samples.items()}
    slopes = [
        (mins[d2] - mins[d1]) / (d2 - d1)
        for i, d1 in enumerate(depths)
        for d2 in depths[i + 1 :]
        if mins[d2] > mins[d1]
    ]
    if slopes:
        per_exec = float(min(slopes))
    else:
        # pathological dispatch noise (non-monotone chain minima): fall back
        # to the most conservative defensible estimate
        dmax = depths[-1]
        per_exec = float(mins[dmax] / dmax)
    t1s = samples[depths[0]]
    _cache["bench"] = {
        "t1": float(mins[depths[0]]),
        "tN": float(mins[depths[-1]]),
        "chain": depths[-1],
        "per_exec": per_exec,
        "mins": mins,
    }

    out = np.empty((B, S, D), np.float32)
    yT_all = np.asarray(out_arrs[out_names.index("yT")]).reshape(8, D, S)
    for b in range(B):
        out[b] = (yT_all[2 * b].astype(np.float32) + yT_all[2 * b + 1].astype(np.float32)).T
    return t1s, out


# revision 39
# speedup vs baseline: 1.0595x; 1.0429x over previous
"""Multi-head causal attention (B=4,S=2048,D=1024,H=16,d=64) on 8 trn2 cores.

Sharding: 8 cores = 4 batches x 2 head-halves (tensor parallel over heads).
Each core handles one batch, 8 heads (4 head-pairs), and ALL 2048 queries.
K/V/Q projections are computed only for the core's own heads, so nothing is
duplicated across cores (the seq-split alternative projects full K/V twice
per batch).  The output projection is row-sharded: each core emits a partial
y^T = Wo^T[own-head rows].T-style contribution and the HOST adds the two
halves per batch (cheap numpy add; no device collectives).  The bias bo is
fed as real data to half-0 cores and zeros to half-1 cores so the host-side
add applies it exactly once.  The device program is identical on all cores;
only input data differs.

On-device structure per core:
  - phase 1: project K,Q for pair-0 chunk 0 up front; everything else is
    emitted just-in-time inside the attention stream (the PE runs its queue
    in emission order, so emission placement is the schedule).
  - phase 2: per head-pair, per 512-wide query chunk, per 128-row k-tile:
    scores^T -> exp (one fused ScalarE activation for both heads of the
    pair) -> causal mask mul on the diagonal block -> AV matmul with a
    ones-column appended to V so softmax denominators fall out of the same
    accumulator.  Remaining V/K/Q projections are interleaved into this
    stream as PE fill work so the PE never idles while ScalarE runs exp.
  - phase 3: output projection y^T = sum_p WoT_rows[p].T @ O^T[p] + bias,
    emitted per 512-query-column group as soon as pair 3's chunk for those
    columns is normalized, so it overlaps the attention tail; bf16 output
    (upcast + summed on host).

Layout tricks (kept from the seq-split version):
  - scores computed transposed S^T[sk, sq]; denominators via ones-column.
  - exp on ScalarE with 1/sqrt(64) folded into the activation scale.
  - all matmul operands bf16 (full PE rate), fp32 PSUM accumulation.
  - PSUM budget: sc [128,1024] x2 (4 banks, double-buffered so exp never
    blocks the next tile's scores) + av [128,512] x2 (2 banks,
    double-buffered across chunks) + proj acc [128,512] x2 = 8 banks.
"""

import numpy as np
import ml_dtypes

B, S, D = 4, 2048, 1024
H, DH = 16, 64
HC = H // 2            # heads per core
NP = HC // 2           # head-pairs per core = 4
NKT = S // 128         # 16 k-tiles
NDC = D // 128         # 8 contraction chunks
NCH = S // 512         # 4 query chunks of 512
VST = 66               # V column stride per head (64 V cols + 1 ones + 1 pad)

BF16 = ml_dtypes.bfloat16

_cache = {}


def _build():
    import concourse.bass as bass
    import concourse.tile as tile
    import concourse.mybir as mybir
    from concourse import bacc
    from contextlib import ExitStack

    dt = mybir.dt
    AF = mybir.ActivationFunctionType

    nc = bacc.Bacc(
        "TRN2",
        target_bir_lowering=False,
        debug=False,
        enable_asserts=False,
        num_devices=8,
    )

    qt_d = nc.dram_tensor("qT", [D, S], dt.bfloat16, kind="ExternalInput").ap()
    kt_d = nc.dram_tensor("kT", [D, S], dt.bfloat16, kind="ExternalInput").ap()
    vt_d = nc.dram_tensor("vT", [D, S], dt.bfloat16, kind="ExternalInput").ap()
    wq_d = nc.dram_tensor("Wq", [NP, 128, NDC, 2, DH], dt.bfloat16, kind="ExternalInput").ap()
    wk_d = nc.dram_tensor("Wk", [NP, 128, NDC, 2, DH], dt.bfloat16, kind="ExternalInput").ap()
    wv_d = nc.dram_tensor("Wv", [NDC, 128, HC, DH], dt.bfloat16, kind="ExternalInput").ap()
    wot_d = nc.dram_tensor("WoT", [NP, 128, D], dt.bfloat16, kind="ExternalInput").ap()
    bo_d = nc.dram_tensor("bo", [128, NDC], dt.float32, kind="ExternalInput").ap()
    mk_d = nc.dram_tensor("mask", [128, 128], dt.bfloat16, kind="ExternalInput").ap()
    y_d = nc.dram_tensor("yT", [D, S], dt.bfloat16, kind="ExternalOutput").ap()

    with tile.TileContext(nc) as tc, ExitStack() as ctx:
        const = ctx.enter_context(tc.tile_pool(name="const", bufs=1))
        work = ctx.enter_context(tc.tile_pool(name="work", bufs=2))
        pp = ctx.enter_context(tc.tile_pool(name="pp", bufs=1, space="PSUM"))

        # ---- persistent SBUF tensors ----------------------------------
        mask = const.tile([128, 128], dt.bfloat16, tag="mask")
        bo_sb = const.tile([128, NDC], dt.float32, tag="bo")
        wv_sb = const.tile([128, NDC, HC, DH], dt.bfloat16, tag="wv")
        wot_sb = [
            const.tile([128, D], dt.bfloat16, tag=f"wot{p}", name=f"wot{p}")
            for p in range(NP)
        ]
        wq_sb = [
            const.tile([128, NDC, 2, DH], dt.bfloat16, tag=f"wq{p}", name=f"wq{p}")
            for p in range(NP)
        ]
        wk_sb = [
            const.tile([128, NDC, 2, DH], dt.bfloat16, tag=f"wk{p}", name=f"wk{p}")
            for p in range(NP)
        ]
        qt_sb = [
            const.tile([128, S], dt.bfloat16, tag=f"qt{p}", name=f"qt{p}")
            for p in range(NP)
        ]
        kt_sb = [
            const.tile([128, S], dt.bfloat16, tag=f"kt{p}", name=f"kt{p}")
            for p in range(NP)
        ]
        v_sb = [
            const.tile([128, HC, VST], dt.bfloat16, tag=f"v{t}", name=f"v{t}")
            for t in range(NKT)
        ]
        ot_sb = [
            const.tile([128, S], dt.bfloat16, tag=f"ot{p}", name=f"ot{p}")
            for p in range(NP)
        ]
        warm = const.tile([1, 8], dt.float32, tag="warm")

        # persistent k-slabs: K projection fills decouple from DMA order
        ktt = [
            const.tile([128, NDC, 512], dt.bfloat16, tag=f"ktt{c}", name=f"ktt{c}")
            for c in range(NCH)
        ]

        # exp table load (~2.7us) kicked off immediately
        nc.vector.memset(warm, 0.0)
        nc.scalar.activation(out=warm, in_=warm, func=AF.Exp, scale=1.0)
        for t in range(NKT):
            nc.vector.memset(v_sb[t][:, :, 64:65], 1.0)
        # tiny dummy matmuls during the initial DMA wait finish the PE
        # p-state ramp before the first real projection arrives
        warm_ps = pp.tile([128, 512], dt.float32, tag="acc", bufs=2, name="warm_ps")
        for _ in range(40):
            nc.tensor.matmul(
                warm_ps[0:1, 0:8], lhsT=warm[:, 0:1], rhs=warm,
                start=True, stop=True,
            )

        def load_slab(src_d, bi):
            """[D, 512] D-major slab -> tt[128, dc, 512] bf16."""
            tt = work.tile([128, NDC, 512], dt.bfloat16, tag="tt", bufs=3)
            nc.sync.dma_start(
                out=tt,
                in_=src_d[:, 512 * bi : 512 * (bi + 1)].rearrange(
                    "(dc p) c -> p dc c", p=128
                ),
            )
            return tt

        def proj_unit(tt, w_sb_p, out_sb_p, bi):
            """One [128, 512] projection: out_sb_p[:, 512bi:...] = W^T @ x."""
            ps = pp.tile([128, 512], dt.float32, tag="acc", bufs=2)
            for dc in range(NDC):
                nc.tensor.matmul(
                    ps,
                    lhsT=w_sb_p[:, dc],
                    rhs=tt[:, dc, :],
                    start=(dc == 0),
                    stop=(dc == NDC - 1),
                )
            nc.vector.tensor_copy(out=out_sb_p[:, 512 * bi : 512 * (bi + 1)], in_=ps)

        def v_unit(tt, bi, tsub):
            """Project V (+ones) for k-tile 4*bi+tsub."""
            kt = 4 * bi + tsub
            ps = pp.tile([128, 512], dt.float32, tag="acc", bufs=2)
            for dc in range(NDC):
                nc.tensor.matmul(
                    ps,
                    lhsT=tt[:, dc, 128 * tsub : 128 * (tsub + 1)],
                    rhs=wv_sb[:, dc],
                    start=(dc == 0),
                    stop=(dc == NDC - 1),
                )
            nc.vector.tensor_copy(
                out=v_sb[kt][:, :, 0:DH],
                in_=ps.rearrange("p (h v) -> p h v", v=DH),
            )

        # ---- phase 0/1 + fill schedule --------------------------------
        # The PE executes its queue in emission order, so emission IS the
        # schedule.  DMA order puts pair0-chunk0's needs first; every other
        # projection unit is emitted just-in-time inside the attention
        # stream, where it doubles as PE fill work during ScalarE exp.
        def dma_wk(p):
            nc.sync.dma_start(out=wk_sb[p], in_=wk_d[p])

        def dma_ktt(c):
            nc.sync.dma_start(
                out=ktt[c],
                in_=kt_d[:, 512 * c : 512 * (c + 1)].rearrange("(dc p) c -> p dc c", p=128),
            )

        dma_ktt(0)
        dma_wk(0)
        dma_wk(1)
        q_first = load_slab(qt_d, 0)
        nc.sync.dma_start(out=wq_sb[0], in_=wq_d[0])
        v_first = load_slab(vt_d, 0)
        nc.sync.dma_start(out=wv_sb, in_=wv_d.rearrange("dc p h v -> p dc h v"))
        nc.sync.dma_start(out=mask, in_=mk_d)
        dma_ktt(1)
        # pair0-chunk0 critical path (K(0,1) fills the q-slab DMA wait)
        proj_unit(ktt[0], wk_sb[0], kt_sb[0], 0)
        proj_unit(ktt[0], wk_sb[1], kt_sb[1], 0)
        proj_unit(q_first, wq_sb[0], qt_sb[0], 0)
        # prefetched q-slabs: slabs[key] holds a loaded tile for (src, c)
        slabs = {("q", 0): q_first, ("v", 0): v_first}
        slabs[("q", 1)] = load_slab(qt_d, 1)
        for c in range(2, NCH):
            dma_ktt(c)
            dma_wk(c)
        slabs[("q", 2)] = load_slab(qt_d, 2)

        def take_slab(src, c):
            tt = slabs.pop((src, c), None)
            if tt is None:
                tt = load_slab(qt_d if src == "q" else vt_d, c)
            return tt

        def mk_prefetch(src, c):
            def go():
                if (src, c) not in slabs:
                    slabs[(src, c)] = load_slab(qt_d if src == "q" else vt_d, c)
            return go

        def mk_v_fill(c):
            def go():
                tt = take_slab("v", c)
                for tsub in range(4):
                    v_unit(tt, c, tsub)
            return go

        def mk_q_fill(c, ps):
            def go():
                tt = take_slab("q", c)
                for p in ps:
                    proj_unit(tt, wq_sb[p], qt_sb[p], c)
            return go

        def mk_k_fill(c, p):
            def go():
                proj_unit(ktt[c], wk_sb[p], kt_sb[p], c)
            return go

        def mk_wq_dma(p):
            def go():
                nc.sync.dma_start(out=wq_sb[p], in_=wq_d[p])
            return go

        fill = {(p, c): [] for p in range(NP) for c in range(NCH)}
        # pair 0: its own K/Q chunks just-in-time + all V projections
        fill[(0, 0)] += [mk_v_fill(0), mk_prefetch("v", 1), mk_wq_dma(1)]
        for c in range(1, NCH):
            fill[(0, c)] += [mk_k_fill(c, 0), mk_q_fill(c, [0]), mk_v_fill(c)]
            if c + 1 < NCH:
                fill[(0, c)].append(mk_prefetch("v", c + 1))
        fill[(0, 2)].append(mk_wq_dma(2))
        fill[(0, 3)].append(mk_prefetch("q", 0))   # reload for pair1/2
        # pair 1: K just-in-time; Q for pairs 1+2 (slab prefetched 1 chunk out)
        for c in range(NCH):
            ks = [] if c == 0 else [mk_k_fill(c, 1)]
            fill[(1, c)] += ks + [mk_q_fill(c, [1, 2])]
            if c + 1 < NCH:
                fill[(1, c)].append(mk_prefetch("q", c + 1))
        fill[(1, 2)].append(mk_wq_dma(3))
        fill[(1, 3)].append(mk_prefetch("q", 0))   # reload for pair3
        # pair 2: K just-in-time; Q for pair 3
        for c in range(NCH):
            fill[(2, c)] += [mk_k_fill(c, 2), mk_q_fill(c, [3])]
            if c + 1 < NCH:
                fill[(2, c)].append(mk_prefetch("q", c + 1))
        # pair 3: its K just-in-time (+ outproj hooks elsewhere)
        for c in range(NCH):
            fill[(3, c)].append(mk_k_fill(c, 3))

        def phase3_dmas():
            nc.sync.dma_start(out=bo_sb, in_=bo_d)
            for p in range(NP):
                nc.sync.dma_start(out=wot_sb[p], in_=wot_d[p])

        fill[(3, 0)].append(phase3_dmas)

        # ---- phase 2: attention ---------------------------------------
        def emit_av(p, avs, prev, c, ntile):
            t, pt = prev
            start = max(0, 128 * t - 512 * c)
            for s in range(2):
                nc.tensor.matmul(
                    avs[s][0:65, start:512],
                    lhsT=v_sb[t][:, 2 * p + s, 0:65],
                    rhs=pt[:, s, start:512],
                    start=(t == 0),
                    stop=(t == ntile - 1),
                )

        pending_norm = [None]

        def mk_norm(p, c, avs):
            def go():
                for s in range(2):
                    po = 64 * s
                    # reciprocal of the denominator row straight out of PSUM,
                    # broadcast to 64 partitions on the (otherwise idle)
                    # GPSIMD engine; the PE stays out of the normalize.
                    rd = work.tile([1, 512], dt.float32, tag="rd", bufs=4)
                    nc.vector.reciprocal(out=rd, in_=avs[s][64:65, :])
                    rb = work.tile([64, 512], dt.float32, tag="rb", bufs=4)
                    nc.gpsimd.partition_broadcast(rb, rd)
                    nc.vector.tensor_mul(
                        ot_sb[p][po : po + 64, 512 * c : 512 * (c + 1)],
                        avs[s][0:64, :],
                        rb,
                    )
            return go

        def outproj_hf(hf):
            """Output projection for query columns [512*hf, 512*hf+512).
            Emitted as soon as every pair's chunk-hf normalize is done, so
            phase 3 overlaps the tail of the attention stream."""
            c0 = 512 * hf
            for dc in range(NDC):
                yp = pp.tile([128, 512], dt.float32, tag="acc", bufs=2)
                for p in range(NP):
                    nc.tensor.matmul(
                        yp,
                        lhsT=wot_sb[p][:, 128 * dc : 128 * (dc + 1)],
                        rhs=ot_sb[p][:, c0 : c0 + 512],
                        start=(p == 0),
                        stop=(p == NP - 1),
                    )
                ys = work.tile([128, 512], dt.bfloat16, tag="ys", bufs=3)
                nc.vector.tensor_scalar_add(ys, yp, bo_sb[:, dc : dc + 1])
                nc.sync.dma_start(
                    out=y_d[128 * dc : 128 * (dc + 1), c0 : c0 + 512], in_=ys
                )

        for p in range(NP):
            for c in range(NCH):
                ntile = 4 * c + 4          # k-tiles 0..4c+3
                for th in fill[(p, c)]:
                    th()
                # previous chunk's bcast+recip+mul, behind the fills so the
                # broadcast matmul runs on a warm PE
                if pending_norm[0] is not None:
                    pending_norm[0]()
                if p == NP - 1 and c >= 1:
                    outproj_hf(c - 1)
                avs = [
                    pp.tile([128, 512], dt.float32, tag="av", bufs=2, name=f"av{s}")
                    for s in range(2)
                ]

                prev = None
                for t in range(ntile):
                    start = max(0, 128 * t - 512 * c)
                    # scores^T for both heads into one PSUM tile
                    sc = pp.tile([128, 1024], dt.float32, tag="sc", bufs=2)
                    for s in range(2):
                        po = 64 * s
                        nc.tensor.matmul(
                            sc[:, 512 * s + start : 512 * (s + 1)],
                            lhsT=kt_sb[p][po : po + 64, 128 * t : 128 * (t + 1)],
                            rhs=qt_sb[p][po : po + 64, 512 * c + start : 512 * (c + 1)],
                            start=True,
                            stop=True,
                        )
                    # fused exp for both heads
                    pt = work.tile([128, 2, 512], dt.bfloat16, tag="pt", bufs=4)
                    nc.scalar.activation(
                        out=pt[:, :, start:512],
                        in_=sc.rearrange("p (s l) -> p s l", s=2)[:, :, start:512],
                        func=AF.Exp,
                        scale=0.125,
                    )
                    if start > 0 or t == 4 * c:
                        # diagonal tile: mask the leading 128 columns
                        for s in range(2):
                            nc.vector.tensor_mul(
                                pt[:, s, start : start + 128],
                                pt[:, s, start : start + 128],
                                mask,
                            )
                    if prev is not None:
                        emit_av(p, avs, prev, c, ntile)
                    prev = (t, pt)
                emit_av(p, avs, prev, c, ntile)

                pending_norm[0] = mk_norm(p, c, avs)

        # final column group: pairs 0-2 pre-accumulated during pair3-c3
        # attention (started above); only pair 3's matmul + bias + dma wait
        # for the last normalize.
        pending_norm[0]()
        outproj_hf(NCH - 1)

    nc.compile()
    return nc


def _get_program():
    if "nc" not in _cache:
        _cache["nc"] = _build()
    return _cache["nc"]


def kernel(q, k, v, Wq, Wk, Wv, Wo, bo, trace=False):
    from concourse.bass_utils import run_bass_kernel_spmd

    nc = _get_program()
    in_maps = _make_in_maps(q, k, v, Wq, Wk, Wv, Wo, bo)
    res = run_bass_kernel_spmd(nc, in_maps, core_ids=list(range(8)), trace=trace)
    _cache["last_results"] = res

    out = np.empty((B, S, D), np.float32)
    for b in range(B):
        out[b] = (
            res.results[2 * b]["yT"].astype(np.float32)
            + res.results[2 * b + 1]["yT"].astype(np.float32)
        ).T
    return out


def last_exec_time_ns():
    res = _cache.get("last_results")
    return getattr(res, "exec_time_ns", None) if res is not None else None


def _make_in_maps(q, k, v, Wq, Wk, Wv, Wo, bo):
    q = np.asarray(q, np.float32)
    k = np.asarray(k, np.float32)
    v = np.asarray(v, np.float32)

    def _pack_qk(W, g):
        # [H, D, DH] half-slice -> [NP, 128, NDC, 2, DH] (2KB DMA runs)
        Wg = np.asarray(W, np.float32)[8 * g : 8 * (g + 1)].astype(BF16)
        return np.ascontiguousarray(
            Wg.reshape(NP, 2, NDC, 128, DH).transpose(0, 3, 2, 1, 4)
        )

    WoT = np.ascontiguousarray(np.asarray(Wo, np.float32).T).astype(BF16)
    bo_fp = np.ascontiguousarray(np.asarray(bo, np.float32).reshape(NDC, 128).T)
    mask = np.triu(np.ones((128, 128), np.float32)).astype(BF16)

    halves = []
    for g in range(2):
        halves.append(
            {
                "Wq": _pack_qk(Wq, g),
                "Wk": _pack_qk(Wk, g),
                "Wv": np.ascontiguousarray(
                    np.asarray(Wv, np.float32)[8 * g : 8 * (g + 1)]
                    .astype(BF16)
                    .transpose(1, 0, 2)
                    .reshape(NDC, 128, HC, DH)
                ),
                "WoT": np.ascontiguousarray(
                    WoT[512 * g : 512 * (g + 1)].reshape(NP, 128, D)
                ),
                "bo": bo_fp if g == 0 else np.zeros_like(bo_fp),
                "mask": mask,
            }
        )

    in_maps = []
    for b in range(B):
        qT = np.ascontiguousarray(q[b].T).astype(BF16)
        kT = np.ascontiguousarray(k[b].T).astype(BF16)
        vT = np.ascontiguousarray(v[b].T).astype(BF16)
        for g in range(2):
            in_maps.append({"qT": qT, "kT": kT, "vT": vT, **halves[g]})
    return in_maps


def benchmark(q, k, v, Wq, Wk, Wv, Wo, bo, iters=20):
    """Steady-state device timing: jit once, keep inputs device-resident,
    time repeated executions.  Returns (per_iter_seconds_list, output)."""
    import time
    import jax
    import jax.numpy as jnp
    import concourse.mybir as mybir
    from jax.experimental.shard_map import shard_map
    from jax.sharding import Mesh, NamedSharding, PartitionSpec
    from concourse import bass2jax

    nc = _get_program()
    bass2jax.install_neuronx_cc_hook()

    in_maps = _make_in_maps(q, k, v, Wq, Wk, Wv, Wo, bo)

    partition_name = nc.partition_id_tensor.name if nc.partition_id_tensor else None
    in_names, out_names, out_avals, zero_shapes = [], [], [], []
    for alloc in nc.m.functions[0].allocations:
        if not isinstance(alloc, mybir.MemoryLocationSet):
            continue
        name = alloc.memorylocations[0].name
        if alloc.kind == "ExternalInput":
            if name != partition_name:
                in_names.append(name)
        elif alloc.kind == "ExternalOutput":
            out_names.append(name)
            shape = tuple(alloc.tensor_shape)
            dtype = mybir.dt.np(alloc.dtype)
            out_avals.append(jax.core.ShapedArray(shape, dtype))
            zero_shapes.append((shape, dtype))
    n_params = len(in_names)
    all_names = in_names + out_names
    if partition_name is not None:
        all_names.append(partition_name)
    donate = tuple(range(n_params, n_params + len(out_names)))

    n_outs = len(out_names)

    def _one(args):
        operands = list(args)
        if partition_name is not None:
            operands.append(bass2jax.partition_id_tensor())
        outs = bass2jax._bass_exec_p.bind(
            *operands,
            out_avals=tuple(out_avals),
            in_names=tuple(all_names),
            out_names=tuple(out_names),
            lowering_input_output_aliases=(),
            sim_require_finite=True,
            sim_require_nnan=True,
            nc=nc,
        )
        return tuple(outs)

    def _body(*args):
        return _one(args)

    devices = jax.devices()[:8]
    mesh = Mesh(np.asarray(devices), ("core",))
    spec = PartitionSpec("core")
    sh = NamedSharding(mesh, spec)
    f1 = jax.jit(
        shard_map(
            _body, mesh=mesh,
            in_specs=(spec,) * (n_params + n_outs),
            out_specs=(spec,) * n_outs,
            check_rep=False,
        ),
        donate_argnums=donate,
        keep_unused=True,
    )
    concat_in = [
        jax.device_put(
            np.concatenate([np.asarray(in_maps[c][nm]) for c in range(8)], axis=0), sh
        )
        for nm in in_names
    ]

    zfns = [
        jax.jit(
            (lambda s, d: (lambda: jnp.zeros((8 * s[0], *s[1:]), d)))(s, d),
            out_shardings=sh,
        )
        for s, d in zero_shapes
    ]

    def make_zeros(n):
        return [[zf() for zf in zfns] for _ in range(n)]

    # warmup (compile)
    out_arrs = f1(*concat_in, *make_zeros(1)[0])
    jax.block_until_ready(out_arrs)

    # slope fit across chain depths, robust to bimodal dispatch latency
    depths = [4, 16, 40]
    samples = {d: [] for d in depths}
    for _ in range(max(iters, 10)):
        for d in depths:
            zsl = make_zeros(d)
            jax.block_until_ready(zsl)
            t0 = time.perf_counter()
            outs = [f1(*concat_in, *zsl[i]) for i in range(d)]
            jax.block_until_ready(outs)
            samples[d].append(time.perf_counter() - t0)
            out_arrs = outs[-1]
    mins = {d: min(v) for d, v in samples.items()}
    slopes = [
        (mins[d2] - mins[d1]) / (d2 - d1)
        for i, d1 in enumerate(depths)
        for d2 in depths[i + 1 :]
        if mins[d2] > mins[d1]
    ]
    if slopes:
        per_exec = float(min(slopes))
    else:
        # pathological dispatch noise (non-monotone chain minima): fall back
        # to the most conservative defensible estimate
        dmax = depths[-1]
        per_exec = float(mins[dmax] / dmax)
    t1s = samples[depths[0]]
    _cache["bench"] = {
        "t1": float(mins[depths[0]]),
        "tN": float(mins[depths[-1]]),
        "chain": depths[-1],
        "per_exec": per_exec,
        "mins": mins,
    }

    out = np.empty((B, S, D), np.float32)
    yT_all = np.asarray(out_arrs[out_names.index("yT")]).reshape(8, D, S)
    for b in range(B):
        out[b] = (yT_all[2 * b].astype(np.float32) + yT_all[2 * b + 1].astype(np.float32)).T
    return t1s, out


# revision 40
# speedup vs baseline: 1.1391x; 1.0751x over previous
"""Multi-head causal attention (B=4,S=2048,D=1024,H=16,d=64) on 8 trn2 cores.

Sharding: 8 cores = 4 batches x 2 head-halves (tensor parallel over heads).
Each core handles one batch, 8 heads (4 head-pairs), and ALL 2048 queries.
K/V/Q projections are computed only for the core's own heads, so nothing is
duplicated across cores (the seq-split alternative projects full K/V twice
per batch).  The output projection is row-sharded: each core emits a partial
y^T = Wo^T[own-head rows].T-style contribution and the HOST adds the two
halves per batch (cheap numpy add; no device collectives).  The bias bo is
fed as real data to half-0 cores and zeros to half-1 cores so the host-side
add applies it exactly once.  The device program is identical on all cores;
only input data differs.

On-device structure per core:
  - phase 1: project K,Q for pair-0 chunk 0 up front; everything else is
    emitted just-in-time inside the attention stream (the PE runs its queue
    in emission order, so emission placement is the schedule).
  - phase 2: per head-pair, per 512-wide query chunk, per 128-row k-tile:
    scores^T -> exp (one fused ScalarE activation for both heads of the
    pair) -> causal mask mul on the diagonal block -> AV matmul with a
    ones-column appended to V so softmax denominators fall out of the same
    accumulator.  Remaining V/K/Q projections are interleaved into this
    stream as PE fill work so the PE never idles while ScalarE runs exp.
  - phase 3: output projection y^T = sum_p WoT_rows[p].T @ O^T[p] + bias,
    emitted per 512-query-column group as soon as pair 3's chunk for those
    columns is normalized, so it overlaps the attention tail; bf16 output
    (upcast + summed on host).

Layout tricks (kept from the seq-split version):
  - scores computed transposed S^T[sk, sq]; denominators via ones-column.
  - exp on ScalarE with 1/sqrt(64) folded into the activation scale.
  - all matmul operands bf16 (full PE rate), fp32 PSUM accumulation.
  - PSUM budget: sc [128,1024] x2 (4 banks, double-buffered so exp never
    blocks the next tile's scores) + av [128,512] x2 (2 banks,
    double-buffered across chunks) + proj acc [128,512] x2 = 8 banks.
"""

import numpy as np
import ml_dtypes

B, S, D = 4, 2048, 1024
H, DH = 16, 64
HC = H // 2            # heads per core
NP = HC // 2           # head-pairs per core = 4
NKT = S // 128         # 16 k-tiles
NDC = D // 128         # 8 contraction chunks
NCH = S // 512         # 4 query chunks of 512
VST = 66               # V column stride per head (64 V cols + 1 ones + 1 pad)

BF16 = ml_dtypes.bfloat16

_cache = {}


def _build():
    import concourse.bass as bass
    import concourse.tile as tile
    import concourse.mybir as mybir
    from concourse import bacc
    from contextlib import ExitStack

    dt = mybir.dt
    AF = mybir.ActivationFunctionType

    nc = bacc.Bacc(
        "TRN2",
        target_bir_lowering=False,
        debug=False,
        enable_asserts=False,
        num_devices=8,
    )

    qt_d = nc.dram_tensor("qT", [D, S], dt.bfloat16, kind="ExternalInput").ap()
    kt_d = nc.dram_tensor("kT", [D, S], dt.bfloat16, kind="ExternalInput").ap()
    vt_d = nc.dram_tensor("vT", [D, S], dt.bfloat16, kind="ExternalInput").ap()
    wq_d = nc.dram_tensor("Wq", [NP, 128, NDC, 2, DH], dt.bfloat16, kind="ExternalInput").ap()
    wk_d = nc.dram_tensor("Wk", [NP, 128, NDC, 2, DH], dt.bfloat16, kind="ExternalInput").ap()
    wv_d = nc.dram_tensor("Wv", [NDC, 128, HC, DH], dt.bfloat16, kind="ExternalInput").ap()
    wot_d = nc.dram_tensor("WoT", [NP, 128, D], dt.bfloat16, kind="ExternalInput").ap()
    bo_d = nc.dram_tensor("bo", [128, NDC], dt.float32, kind="ExternalInput").ap()
    mk_d = nc.dram_tensor("mask", [128, 128], dt.bfloat16, kind="ExternalInput").ap()
    y_d = nc.dram_tensor("yT", [D, S], dt.bfloat16, kind="ExternalOutput").ap()

    with tile.TileContext(nc) as tc, ExitStack() as ctx:
        const = ctx.enter_context(tc.tile_pool(name="const", bufs=1))
        work = ctx.enter_context(tc.tile_pool(name="work", bufs=2))
        pp = ctx.enter_context(tc.tile_pool(name="pp", bufs=1, space="PSUM"))

        # ---- persistent SBUF tensors ----------------------------------
        mask = const.tile([128, 128], dt.bfloat16, tag="mask")
        bo_sb = const.tile([128, NDC], dt.float32, tag="bo")
        wv_sb = const.tile([128, NDC, HC, DH], dt.bfloat16, tag="wv")
        wot_sb = [
            const.tile([128, D], dt.bfloat16, tag=f"wot{p}", name=f"wot{p}")
            for p in range(NP)
        ]
        wq_sb = [
            const.tile([128, NDC, 2, DH], dt.bfloat16, tag=f"wq{p}", name=f"wq{p}")
            for p in range(NP)
        ]
        wk_sb = [
            const.tile([128, NDC, 2, DH], dt.bfloat16, tag=f"wk{p}", name=f"wk{p}")
            for p in range(NP)
        ]
        qt_sb = [
            const.tile([128, S], dt.bfloat16, tag=f"qt{p}", name=f"qt{p}")
            for p in range(NP)
        ]
        kt_sb = [
            const.tile([128, S], dt.bfloat16, tag=f"kt{p}", name=f"kt{p}")
            for p in range(NP)
        ]
        v_sb = [
            const.tile([128, HC, VST], dt.bfloat16, tag=f"v{t}", name=f"v{t}")
            for t in range(NKT)
        ]
        ot_sb = [
            const.tile([128, S], dt.bfloat16, tag=f"ot{p}", name=f"ot{p}")
            for p in range(NP)
        ]
        warm = const.tile([1, 8], dt.float32, tag="warm")

        # persistent k-slabs: K projection fills decouple from DMA order
        ktt = [
            const.tile([128, NDC, 512], dt.bfloat16, tag=f"ktt{c}", name=f"ktt{c}")
            for c in range(NCH)
        ]

        # exp table load (~2.7us) kicked off immediately
        nc.vector.memset(warm, 0.0)
        nc.scalar.activation(out=warm, in_=warm, func=AF.Exp, scale=1.0)
        for t in range(NKT):
            nc.vector.memset(v_sb[t][:, :, 64:65], 1.0)
        # tiny dummy matmuls during the initial DMA wait finish the PE
        # p-state ramp before the first real projection arrives
        warm_ps = pp.tile([128, 512], dt.float32, tag="acc", bufs=2, name="warm_ps")
        for _ in range(40):
            nc.tensor.matmul(
                warm_ps[0:1, 0:8], lhsT=warm[:, 0:1], rhs=warm,
                start=True, stop=True,
            )

        def load_slab(src_d, bi):
            """[D, 512] D-major slab -> tt[128, dc, 512] bf16."""
            tt = work.tile([128, NDC, 512], dt.bfloat16, tag="tt", bufs=3)
            nc.sync.dma_start(
                out=tt,
                in_=src_d[:, 512 * bi : 512 * (bi + 1)].rearrange(
                    "(dc p) c -> p dc c", p=128
                ),
            )
            return tt

        def proj_unit(tt, w_sb_p, out_sb_p, bi):
            """One [128, 512] projection: out_sb_p[:, 512bi:...] = W^T @ x."""
            ps = pp.tile([128, 512], dt.float32, tag="acc", bufs=2)
            for dc in range(NDC):
                nc.tensor.matmul(
                    ps,
                    lhsT=w_sb_p[:, dc],
                    rhs=tt[:, dc, :],
                    start=(dc == 0),
                    stop=(dc == NDC - 1),
                )
            nc.vector.tensor_copy(out=out_sb_p[:, 512 * bi : 512 * (bi + 1)], in_=ps)

        def v_unit(tt, bi, tsub):
            """Project V (+ones) for k-tile 4*bi+tsub."""
            kt = 4 * bi + tsub
            ps = pp.tile([128, 512], dt.float32, tag="acc", bufs=2)
            for dc in range(NDC):
                nc.tensor.matmul(
                    ps,
                    lhsT=tt[:, dc, 128 * tsub : 128 * (tsub + 1)],
                    rhs=wv_sb[:, dc],
                    start=(dc == 0),
                    stop=(dc == NDC - 1),
                )
            nc.vector.tensor_copy(
                out=v_sb[kt][:, :, 0:DH],
                in_=ps.rearrange("p (h v) -> p h v", v=DH),
            )

        # ---- phase 0/1 + fill schedule --------------------------------
        # The PE executes its queue in emission order, so emission IS the
        # schedule.  DMA order puts pair0-chunk0's needs first; every other
        # projection unit is emitted just-in-time inside the attention
        # stream, where it doubles as PE fill work during ScalarE exp.
        def dma_wk(p):
            nc.sync.dma_start(out=wk_sb[p], in_=wk_d[p])

        def dma_ktt(c):
            nc.sync.dma_start(
                out=ktt[c],
                in_=kt_d[:, 512 * c : 512 * (c + 1)].rearrange("(dc p) c -> p dc c", p=128),
            )

        dma_ktt(0)
        dma_wk(0)
        dma_wk(1)
        q_first = load_slab(qt_d, 0)
        nc.sync.dma_start(out=wq_sb[0], in_=wq_d[0])
        v_first = load_slab(vt_d, 0)
        nc.sync.dma_start(out=wv_sb, in_=wv_d.rearrange("dc p h v -> p dc h v"))
        nc.sync.dma_start(out=mask, in_=mk_d)
        dma_ktt(1)
        # pair0-chunk0 critical path (K(0,1) fills the q-slab DMA wait)
        proj_unit(ktt[0], wk_sb[0], kt_sb[0], 0)
        proj_unit(ktt[0], wk_sb[1], kt_sb[1], 0)
        proj_unit(q_first, wq_sb[0], qt_sb[0], 0)
        # prefetched q-slabs: slabs[key] holds a loaded tile for (src, c)
        slabs = {("q", 0): q_first, ("v", 0): v_first}
        slabs[("q", 1)] = load_slab(qt_d, 1)
        for c in range(2, NCH):
            dma_ktt(c)
            dma_wk(c)
        slabs[("q", 2)] = load_slab(qt_d, 2)

        def take_slab(src, c):
            tt = slabs.pop((src, c), None)
            if tt is None:
                tt = load_slab(qt_d if src == "q" else vt_d, c)
            return tt

        def mk_prefetch(src, c):
            def go():
                if (src, c) not in slabs:
                    slabs[(src, c)] = load_slab(qt_d if src == "q" else vt_d, c)
            return go

        def mk_v_fill(c):
            def go():
                tt = take_slab("v", c)
                for tsub in range(4):
                    v_unit(tt, c, tsub)
            return go

        def mk_q_fill(c, ps):
            def go():
                tt = take_slab("q", c)
                for p in ps:
                    proj_unit(tt, wq_sb[p], qt_sb[p], c)
            return go

        def mk_k_fill(c, p):
            def go():
                proj_unit(ktt[c], wk_sb[p], kt_sb[p], c)
            return go

        def mk_wq_dma(p):
            def go():
                nc.sync.dma_start(out=wq_sb[p], in_=wq_d[p])
            return go

        fill = {(p, c): [] for p in range(NP) for c in range(NCH)}
        # pair 0: its own K/Q chunks just-in-time + all V projections
        fill[(0, 0)] += [mk_v_fill(0), mk_prefetch("v", 1), mk_wq_dma(1)]
        for c in range(1, NCH):
            fill[(0, c)] += [mk_k_fill(c, 0), mk_q_fill(c, [0]), mk_v_fill(c)]
            if c + 1 < NCH:
                fill[(0, c)].append(mk_prefetch("v", c + 1))
        fill[(0, 2)].append(mk_wq_dma(2))
        fill[(0, 3)].append(mk_prefetch("q", 0))   # reload for pair1/2
        # pair 1: K just-in-time; Q for pairs 1+2 (slab prefetched 1 chunk out)
        for c in range(NCH):
            ks = [] if c == 0 else [mk_k_fill(c, 1)]
            fill[(1, c)] += ks + [mk_q_fill(c, [1, 2])]
            if c + 1 < NCH:
                fill[(1, c)].append(mk_prefetch("q", c + 1))
        fill[(1, 2)].append(mk_wq_dma(3))
        fill[(1, 3)].append(mk_prefetch("q", 0))   # reload for pair3
        # pair 2: K just-in-time; Q for pair 3
        for c in range(NCH):
            fill[(2, c)] += [mk_k_fill(c, 2), mk_q_fill(c, [3])]
            if c + 1 < NCH:
                fill[(2, c)].append(mk_prefetch("q", c + 1))
        # pair 3: its K just-in-time (+ outproj hooks elsewhere)
        for c in range(NCH):
            fill[(3, c)].append(mk_k_fill(c, 3))

        def phase3_dmas():
            nc.sync.dma_start(out=bo_sb, in_=bo_d)
            for p in range(NP):
                nc.sync.dma_start(out=wot_sb[p], in_=wot_d[p])

        fill[(3, 0)].append(phase3_dmas)

        # ---- phase 2: attention ---------------------------------------
        def emit_av(p, avs, prev, c, ntile):
            t, pt = prev
            start = max(0, 128 * t - 512 * c)
            for s in range(2):
                nc.tensor.matmul(
                    avs[s][0:65, start:512],
                    lhsT=v_sb[t][:, 2 * p + s, 0:65],
                    rhs=pt[:, s, start:512],
                    start=(t == 0),
                    stop=(t == ntile - 1),
                )

        pending_norm = [None]

        def mk_norm(p, c, avs):
            def go():
                # reciprocal of the denominator rows straight out of PSUM,
                # broadcast to 64 partitions on the (otherwise idle) GPSIMD
                # engine; both heads' recips are emitted before the muls so
                # the in-order DVE queue never stalls on the GPSIMD hop.
                rds, rbs = [], []
                for s in range(2):
                    rd = work.tile([1, 512], dt.float32, tag="rd", bufs=4)
                    nc.vector.reciprocal(out=rd, in_=avs[s][64:65, :])
                    rds.append(rd)
                for s in range(2):
                    rb = work.tile([64, 512], dt.float32, tag="rb", bufs=4)
                    nc.gpsimd.partition_broadcast(rb, rds[s])
                    rbs.append(rb)
                for s in range(2):
                    nc.vector.tensor_mul(
                        ot_sb[p][64 * s : 64 * s + 64, 512 * c : 512 * (c + 1)],
                        avs[s][0:64, :],
                        rbs[s],
                    )
            return go

        def outproj_hf(hf):
            """Output projection for query columns [512*hf, 512*hf+512).
            Emitted as soon as every pair's chunk-hf normalize is done, so
            phase 3 overlaps the tail of the attention stream."""
            c0 = 512 * hf
            for dc in range(NDC):
                yp = pp.tile([128, 512], dt.float32, tag="acc", bufs=2)
                for p in range(NP):
                    nc.tensor.matmul(
                        yp,
                        lhsT=wot_sb[p][:, 128 * dc : 128 * (dc + 1)],
                        rhs=ot_sb[p][:, c0 : c0 + 512],
                        start=(p == 0),
                        stop=(p == NP - 1),
                    )
                ys = work.tile([128, 512], dt.bfloat16, tag="ys", bufs=3)
                nc.vector.tensor_scalar_add(ys, yp, bo_sb[:, dc : dc + 1])
                nc.sync.dma_start(
                    out=y_d[128 * dc : 128 * (dc + 1), c0 : c0 + 512], in_=ys
                )

        for p in range(NP):
            for c in range(NCH):
                ntile = 4 * c + 4          # k-tiles 0..4c+3
                for th in fill[(p, c)]:
                    th()
                # previous chunk's bcast+recip+mul, behind the fills so the
                # broadcast matmul runs on a warm PE
                if pending_norm[0] is not None:
                    pending_norm[0]()
                if p == NP - 1 and c >= 1:
                    outproj_hf(c - 1)
                avs = [
                    pp.tile([128, 512], dt.float32, tag="av", bufs=2, name=f"av{s}")
                    for s in range(2)
                ]

                prev = None
                for t in range(ntile):
                    start = max(0, 128 * t - 512 * c)
                    # scores^T for both heads into one PSUM tile
                    sc = pp.tile([128, 1024], dt.float32, tag="sc", bufs=2)
                    for s in range(2):
                        po = 64 * s
                        nc.tensor.matmul(
                            sc[:, 512 * s + start : 512 * (s + 1)],
                            lhsT=kt_sb[p][po : po + 64, 128 * t : 128 * (t + 1)],
                            rhs=qt_sb[p][po : po + 64, 512 * c + start : 512 * (c + 1)],
                            start=True,
                            stop=True,
                        )
                    # fused exp for both heads
                    pt = work.tile([128, 2, 512], dt.bfloat16, tag="pt", bufs=4)
                    nc.scalar.activation(
                        out=pt[:, :, start:512],
                        in_=sc.rearrange("p (s l) -> p s l", s=2)[:, :, start:512],
                        func=AF.Exp,
                        scale=0.125,
                    )
                    if start > 0 or t == 4 * c:
                        # diagonal tile: mask the leading 128 columns
                        for s in range(2):
                            nc.vector.tensor_mul(
                                pt[:, s, start : start + 128],
                                pt[:, s, start : start + 128],
                                mask,
                            )
                    if prev is not None:
                        emit_av(p, avs, prev, c, ntile)
                    prev = (t, pt)
                emit_av(p, avs, prev, c, ntile)

                pending_norm[0] = mk_norm(p, c, avs)

        # final column group: pairs 0-2 pre-accumulated during pair3-c3
        # attention (started above); only pair 3's matmul + bias + dma wait
        # for the last normalize.
        pending_norm[0]()
        outproj_hf(NCH - 1)

    nc.compile()
    return nc


def _get_program():
    if "nc" not in _cache:
        _cache["nc"] = _build()
    return _cache["nc"]


def kernel(q, k, v, Wq, Wk, Wv, Wo, bo, trace=False):
    from concourse.bass_utils import run_bass_kernel_spmd

    nc = _get_program()
    in_maps = _make_in_maps(q, k, v, Wq, Wk, Wv, Wo, bo)
    res = run_bass_kernel_spmd(nc, in_maps, core_ids=list(range(8)), trace=trace)
    _cache["last_results"] = res

    out = np.empty((B, S, D), np.float32)
    for b in range(B):
        out[b] = (
            res.results[2 * b]["yT"].astype(np.float32)
            + res.results[2 * b + 1]["yT"].astype(np.float32)
        ).T
    return out


def last_exec_time_ns():
    res = _cache.get("last_results")
    return getattr(res, "exec_time_ns", None) if res is not None else None


def _make_in_maps(q, k, v, Wq, Wk, Wv, Wo, bo):
    q = np.asarray(q, np.float32)
    k = np.asarray(k, np.float32)
    v = np.asarray(v, np.float32)

    def _pack_qk(W, g):
        # [H, D, DH] half-slice -> [NP, 128, NDC, 2, DH] (2KB DMA runs)
        Wg = np.asarray(W, np.float32)[8 * g : 8 * (g + 1)].astype(BF16)
        return np.ascontiguousarray(
            Wg.reshape(NP, 2, NDC, 128, DH).transpose(0, 3, 2, 1, 4)
        )

    WoT = np.ascontiguousarray(np.asarray(Wo, np.float32).T).astype(BF16)
    bo_fp = np.ascontiguousarray(np.asarray(bo, np.float32).reshape(NDC, 128).T)
    mask = np.triu(np.ones((128, 128), np.float32)).astype(BF16)

    halves = []
    for g in range(2):
        halves.append(
            {
                "Wq": _pack_qk(Wq, g),
                "Wk": _pack_qk(Wk, g),
                "Wv": np.ascontiguousarray(
                    np.asarray(Wv, np.float32)[8 * g : 8 * (g + 1)]
                    .astype(BF16)
                    .transpose(1, 0, 2)
                    .reshape(NDC, 128, HC, DH)
                ),
                "WoT": np.ascontiguousarray(
                    WoT[512 * g : 512 * (g + 1)].reshape(NP, 128, D)
                ),
                "bo": bo_fp if g == 0 else np.zeros_like(bo_fp),
                "mask": mask,
            }
        )

    in_maps = []
    for b in range(B):
        qT = np.ascontiguousarray(q[b].T).astype(BF16)
        kT = np.ascontiguousarray(k[b].T).astype(BF16)
        vT = np.ascontiguousarray(v[b].T).astype(BF16)
        for g in range(2):
            in_maps.append({"qT": qT, "kT": kT, "vT": vT, **halves[g]})
    return in_maps


def benchmark(q, k, v, Wq, Wk, Wv, Wo, bo, iters=20):
    """Steady-state device timing: jit once, keep inputs device-resident,
    time repeated executions.  Returns (per_iter_seconds_list, output)."""
    import time
    import jax
    import jax.numpy as jnp
    import concourse.mybir as mybir
    from jax.experimental.shard_map import shard_map
    from jax.sharding import Mesh, NamedSharding, PartitionSpec
    from concourse import bass2jax

    nc = _get_program()
    bass2jax.install_neuronx_cc_hook()

    in_maps = _make_in_maps(q, k, v, Wq, Wk, Wv, Wo, bo)

    partition_name = nc.partition_id_tensor.name if nc.partition_id_tensor else None
    in_names, out_names, out_avals, zero_shapes = [], [], [], []
    for alloc in nc.m.functions[0].allocations:
        if not isinstance(alloc, mybir.MemoryLocationSet):
            continue
        name = alloc.memorylocations[0].name
        if alloc.kind == "ExternalInput":
            if name != partition_name:
                in_names.append(name)
        elif alloc.kind == "ExternalOutput":
            out_names.append(name)
            shape = tuple(alloc.tensor_shape)
            dtype = mybir.dt.np(alloc.dtype)
            out_avals.append(jax.core.ShapedArray(shape, dtype))
            zero_shapes.append((shape, dtype))
    n_params = len(in_names)
    all_names = in_names + out_names
    if partition_name is not None:
        all_names.append(partition_name)
    donate = tuple(range(n_params, n_params + len(out_names)))

    n_outs = len(out_names)

    def _one(args):
        operands = list(args)
        if partition_name is not None:
            operands.append(bass2jax.partition_id_tensor())
        outs = bass2jax._bass_exec_p.bind(
            *operands,
            out_avals=tuple(out_avals),
            in_names=tuple(all_names),
            out_names=tuple(out_names),
            lowering_input_output_aliases=(),
            sim_require_finite=True,
            sim_require_nnan=True,
            nc=nc,
        )
        return tuple(outs)

    def _body(*args):
        return _one(args)

    devices = jax.devices()[:8]
    mesh = Mesh(np.asarray(devices), ("core",))
    spec = PartitionSpec("core")
    sh = NamedSharding(mesh, spec)
    f1 = jax.jit(
        shard_map(
            _body, mesh=mesh,
            in_specs=(spec,) * (n_params + n_outs),
            out_specs=(spec,) * n_outs,
            check_rep=False,
        ),
        donate_argnums=donate,
        keep_unused=True,
    )
    concat_in = [
        jax.device_put(
            np.concatenate([np.asarray(in_maps[c][nm]) for c in range(8)], axis=0), sh
        )
        for nm in in_names
    ]

    zfns = [
        jax.jit(
            (lambda s, d: (lambda: jnp.zeros((8 * s[0], *s[1:]), d)))(s, d),
            out_shardings=sh,
        )
        for s, d in zero_shapes
    ]

    def make_zeros(n):
        return [[zf() for zf in zfns] for _ in range(n)]

    # warmup (compile)
    out_arrs = f1(*concat_in, *make_zeros(1)[0])
    jax.block_until_ready(out_arrs)

    # slope fit across chain depths, robust to bimodal dispatch latency
    depths = [4, 16, 40]
    samples = {d: [] for d in depths}
    for _ in range(max(iters, 10)):
        for d in depths:
            zsl = make_zeros(d)
            jax.block_until_ready(zsl)
            t0 = time.perf_counter()
            outs = [f1(*concat_in, *zsl[i]) for i in range(d)]
            jax.block_until_ready(outs)
            samples[d].append(time.perf_counter() - t0)
            out_arrs = outs[-1]
    mins = {d: min(v) for d, v in samples.items()}
    slopes = [
        (mins[d2] - mins[d1]) / (d2 - d1)
        for i, d1 in enumerate(depths)
        for d2 in depths[i + 1 :]
        if mins[d2] > mins[d1]
    ]
    if slopes:
        per_exec = float(min(slopes))
    else:
        # pathological dispatch noise (non-monotone chain minima): fall back
        # to the most conservative defensible estimate
        dmax = depths[-1]
        per_exec = float(mins[dmax] / dmax)
    t1s = samples[depths[0]]
    _cache["bench"] = {
        "t1": float(mins[depths[0]]),
        "tN": float(mins[depths[-1]]),
        "chain": depths[-1],
        "per_exec": per_exec,
        "mins": mins,
    }

    out = np.empty((B, S, D), np.float32)
    yT_all = np.asarray(out_arrs[out_names.index("yT")]).reshape(8, D, S)
    for b in range(B):
        out[b] = (yT_all[2 * b].astype(np.float32) + yT_all[2 * b + 1].astype(np.float32)).T
    return t1s, out
